# revision 2
# baseline (speedup 1.0000x reference)
"""GAT layer (nn_GATLayer) on 8 Trainium2 NeuronCores — Bass/Tile kernel.

Contract: kernel(**inputs) takes the FULL inputs of reference.setup_inputs()
  h   [4, 4096, 256]  float32
  adj [4, 4096, 4096] int32 ({0,1})
  W   [256, 128]      float32
  a   [256, 1]        float32
and returns the FULL output [4, 4096, 128] float32.

Sharding: data-parallel over batch (4 batches x 2 cores each); within a
batch the NxN attention is sharded over query rows (2048 rows per core,
flash-attention-style row blocks). Each core computes a partial
pre-activation aggregate out2[i,k] = sum_{j in block} att[j,k] Wh[j,i]; the
host sums the two partials per batch and applies the final ELU.

Device algorithm per core (all matmuls bf16, elementwise bf16 at 4x DVE):
  exp(leakyrelu(s1_j+s2_k)) = e^{s1_j} * max(c_j q_k, p_k)
  with p = e^{s2}, q = e^{0.2 s2}, c_j = e^{-0.8 s1_j}; the e^{s1_j} row
  factor cancels in the softmax (numerator and denominator), so the kernel
  never applies it:
    t[j,k]   = max(c_j * q_k, p_k)            (DVE stt, 4x bf16)
    E[j,k]   = t * mask;  den_j = sum_k E     (DVE stt fused accum, 4x)
    out[i,k]+= sum_j (Wh[j,i]/den_j) E[j,k]   (PE, bf16, PSUM accum)
  mask = adj as bf16 {0,1}: cast on ACT ('A') / Pool ('P') per-tile, or
  'V' tiles use affine_mul_reduce reading uint8 adj directly (1x DVE).
"""
import sys
import os

sys.path.insert(0, "/opt/trn_rl_repo")

import numpy as np
import ml_dtypes

B, N, FIN, FOUT = 4, 4096, 256, 128
JB = N // 2            # rows per core
NJT = JB // 128        # 16 j-tiles per core
NFC = FIN // 128       # fin chunks
KC = 512
NKC = N // KC
W2 = FOUT + 2          # waug cols per fin chunk: [Wh | Wa1 | Wa2]
WS = FOUT + 1          # whaug sbuf cols per j-tile: [Wh | s1]
#        0123456789012345
CONV = list("VAPAPPAPVAPAPPAV")
ADJ_BUFS, M_BUFS, T_BUFS, E_BUFS = 3, 3, 3, 3

_COMPILED = {}


def _build():
    import concourse.bacc as bacc
    import concourse.tile as tile
    from concourse import mybir

    dt = mybir.dt
    ALU = mybir.AluOpType
    AF = mybir.ActivationFunctionType

    nc = bacc.Bacc("TRN2", target_bir_lowering=False, debug=False)
    hT_d = nc.dram_tensor("hT", (FIN, N), dt.bfloat16, kind="ExternalInput").ap()
    adj_d = nc.dram_tensor("adj", (JB, N), dt.uint8, kind="ExternalInput").ap()
    waug_d = nc.dram_tensor("waug", (128, NFC * W2), dt.bfloat16,
                            kind="ExternalInput").ap()
    out_d = nc.dram_tensor("out2", (FOUT, N), dt.float32, kind="ExternalOutput").ap()

    from contextlib import ExitStack

    with tile.TileContext(nc) as tc, ExitStack() as ctx:
        pp = ctx.enter_context(tc.tile_pool(name="persist", bufs=1))
        waug_t = pp.tile([128, NFC * W2], dt.bfloat16)
        nc.sync.dma_start(waug_t[:], waug_d[:])
        hT = [pp.tile([128, N], dt.bfloat16, name=f"hT{c}") for c in range(NFC)]
        for c in range(NFC):
            nc.sync.dma_start(hT[c][:], hT_d[c * 128:(c + 1) * 128, :])

        whaug = pp.tile([128, NJT * WS], dt.float32)
        c_all = pp.tile([128, NJT], dt.float32)
        p_row = pp.tile([1, N], dt.bfloat16)
        q_row = pp.tile([1, N], dt.bfloat16)
        p_b = pp.tile([128, N], dt.bfloat16)
        q_b = pp.tile([128, N], dt.bfloat16)

        # ---------------- prep ----------------
        with tc.tile_pool(name="prps", bufs=2, space="PSUM") as prps:
            # s2 row (chunked) -> p = exp(s2), q = exp(.2 s2) -> broadcasts
            for kc in range(NKC):
                ps = prps.tile([1, KC], dt.float32, tag="srps")
                for c in range(NFC):
                    nc.tensor.matmul(
                        ps[:], waug_t[:, c * W2 + FOUT + 1: c * W2 + W2],
                        hT[c][:, kc * KC:(kc + 1) * KC],
                        start=(c == 0), stop=(c == NFC - 1))
                sl = slice(kc * KC, (kc + 1) * KC)
                nc.scalar.activation(p_row[:, sl], ps[:], AF.Exp,
                                     bias=0.0, scale=1.0)
                nc.scalar.activation(q_row[:, sl], ps[:], AF.Exp,
                                     bias=0.0, scale=0.2)
            nc.gpsimd.partition_broadcast(p_b[:], p_row[:])
            nc.gpsimd.partition_broadcast(q_b[:], q_row[:])

            # Wh_aug own block ([Wh | s1] per j-tile; own rows are hT
            # columns [0, JB) — host rotates the k-axis per core)
            for jt in range(NJT):
                ps = prps.tile([128, WS], dt.float32, tag="whps")
                for c in range(NFC):
                    nc.tensor.matmul(
                        ps[:], hT[c][:, jt * 128:(jt + 1) * 128],
                        waug_t[:, c * W2:c * W2 + WS],
                        start=(c == 0), stop=(c == NFC - 1))
                nc.vector.tensor_copy(whaug[:, jt * WS:(jt + 1) * WS], ps[:])

            s1_view = whaug[:, FOUT::WS]
            nc.scalar.activation(c_all[:], s1_view, AF.Exp, bias=0.0, scale=-0.8)

        # ---------------- main loop ----------------
        with tc.tile_pool(name="adjp", bufs=ADJ_BUFS) as adjp, \
             tc.tile_pool(name="mp", bufs=M_BUFS) as mp, \
             tc.tile_pool(name="tp", bufs=T_BUFS) as tp, \
             tc.tile_pool(name="ep", bufs=E_BUFS) as ep, \
             tc.tile_pool(name="sc", bufs=4) as sc, \
             tc.tile_pool(name="mainps", bufs=1, space="PSUM") as mps:
            psum_out = [mps.tile([128, KC], dt.float32, name=f"pso{k}", tag=f"ps{k}")
                        for k in range(NKC)]

            for jt in range(NJT):
                adjt = adjp.tile([128, N], dt.uint8, tag="adj")
                nc.sync.dma_start(adjt[:], adj_d[jt * 128:(jt + 1) * 128, :])
                t = tp.tile([128, N], dt.bfloat16, tag="t")
                nc.vector.scalar_tensor_tensor(
                    t[:], q_b[:], c_all[:, jt:jt + 1], p_b[:], ALU.mult, ALU.max)
                den = sc.tile([128, 1], dt.float32, tag="den")
                E = ep.tile([128, N], dt.bfloat16, tag="E")
                if CONV[jt] == "V":
                    nc.vector.affine_mul_reduce(E[:], den[:], t[:], adjt[:],
                                                1.0, 0.0)
                else:
                    M = mp.tile([128, N], dt.bfloat16, tag="M")
                    if CONV[jt] == "A":
                        nc.scalar.activation(M[:], adjt[:], AF.Copy,
                                             bias=0.0, scale=1.0)
                    else:
                        nc.gpsimd.tensor_copy(M[:], adjt[:])
                    nc.vector.scalar_tensor_tensor(
                        E[:], t[:], 1.0, M[:], ALU.mult, ALU.mult,
                        accum_out=den[:])
                dinv = sc.tile([128, 1], dt.float32, tag="dinv")
                nc.vector.reciprocal(dinv[:], den[:])
                whp = sc.tile([128, FOUT], dt.bfloat16, tag="whp")
                nc.scalar.activation(whp[:], whaug[:, jt * WS: jt * WS + FOUT],
                                     AF.Copy, bias=0.0, scale=dinv[:, 0:1])
                for kc in range(NKC):
                    nc.tensor.matmul(psum_out[kc][:], whp[:],
                                     E[:, kc * KC:(kc + 1) * KC],
                                     start=(jt == 0), stop=(jt == NJT - 1))

            for kc in range(NKC):
                o = sc.tile([128, KC], dt.float32, tag="drain")
                if kc % 2 == 0:
                    nc.vector.tensor_copy(o[:], psum_out[kc][:])
                else:
                    nc.scalar.copy(o[:], psum_out[kc][:])
                nc.sync.dma_start(out_d[:, kc * KC:(kc + 1) * KC], o[:])

    nc.compile()
    return nc


def _get_nc():
    if "nc" not in _COMPILED:
        _COMPILED["nc"] = _build()
    return _COMPILED["nc"]


def _core_inputs(h_b, adj_b, waug, j0):
    """Per-core input dict. Rotates the k-axis by -j0 so the core's own
    j-block always occupies columns [0, JB) (one SPMD program for all)."""
    hT = np.ascontiguousarray(h_b.T)
    if j0:
        hT = np.ascontiguousarray(np.roll(hT, -j0, axis=1))
    blk = adj_b[j0:j0 + JB]
    if j0:
        blk = np.roll(blk, -j0, axis=1)
    return {
        "hT": hT.astype(ml_dtypes.bfloat16),
        "adj": np.ascontiguousarray(blk).astype(np.uint8),
        "waug": waug,
    }


def kernel(h, adj, W, a):
    from concourse.bass_utils import run_bass_kernel_spmd

    h = np.asarray(h, dtype=np.float32)
    adj = np.asarray(adj)
    W = np.asarray(W, dtype=np.float32)
    a = np.asarray(a, dtype=np.float32)

    # fold attention vector into weights: [W | W@a1 | W@a2], swizzled so fin
    # chunk c occupies columns [c*W2, (c+1)*W2)
    waug = np.concatenate([W, W @ a[:FOUT], W @ a[FOUT:]], axis=1).astype(np.float32)
    waug = np.ascontiguousarray(
        waug.reshape(NFC, 128, W2).transpose(1, 0, 2).reshape(128, NFC * W2)
    ).astype(ml_dtypes.bfloat16)

    nc = _get_nc()
    in_maps = []
    for core in range(8):
        b, half = core // 2, core % 2
        in_maps.append(_core_inputs(h[b], adj[b], waug, half * JB))

    res = run_bass_kernel_spmd(nc, in_maps, list(range(8))).results

    out = np.empty((B, N, FOUT), dtype=np.float32)
    for b in range(B):
        p0 = res[2 * b]["out2"]                       # [FOUT, N], k-order of core 2b (j0=0)
        p1 = np.roll(res[2 * b + 1]["out2"], JB, axis=1)  # undo k rotation
        hp = (p0 + p1).T                              # [N, FOUT]
        out[b] = np.where(hp > 0, hp, np.expm1(np.minimum(hp, 0.0)))
    return out


if __name__ == "__main__":
    # smoke test with random data
    rng = np.random.default_rng(0)
    h = rng.standard_normal((B, N, FIN)).astype(np.float32)
    adj = rng.integers(0, 2, (B, N, N)).astype(np.int32)
    W = (rng.uniform(-1, 1, (FIN, FOUT)) * 0.177).astype(np.float32)
    a = (rng.uniform(-1, 1, (2 * FOUT, 1)) * 0.216).astype(np.float32)
    out = kernel(h=h, adj=adj, W=W, a=a)
    print("out", out.shape, out.dtype, np.abs(out).mean())
